# revision 1
# baseline (speedup 1.0000x reference)
"""Sparse-attention (PVT-style SRA) kernel for 8 Trainium2 NeuronCores.

Sharding: 8 cores = 2 batches x 4 row-quarters of N=8000. Each core computes
its 2000 output rows end-to-end; the spatial-reduction branch (conv+LN+kv) is
replicated per batch. All matmuls run as fp32r (tf32-like) on the PE; the
depthwise conv runs on the vector engine with per-partition tap scalars; the
trilinear upsample is expressed as a per-core interpolation matmul whose
weights (products of 1/4 and 3/4) are fp32r-exact.
"""

import sys

sys.path.insert(0, "/opt/trn_rl_repo")

import contextlib
import numpy as np
import concourse.bacc as bacc
import concourse.mybir as mybir
from concourse.tile import TileContext
from concourse.bass_utils import run_bass_kernel_spmd

dt = mybir.dt
Alu = mybir.AluOpType
Act = mybir.ActivationFunctionType

P = 128
B, N, C = 2, 8000, 512
H, HD = 8, 64
D3 = 20          # full spatial edge (D=H=W)
DR = 10          # reduced spatial edge
NSR = 1000       # DR**3
CT = 4           # C // P
NCHUNK = 2000    # output rows per core
RC = 4           # row chunks per core
RCW = 500        # rows per chunk
MT = 8           # key tiles
MTW = 125        # keys per tile
SCALE = HD ** -0.5
EPS = 1e-6

_PROGRAM = None
TRACE = False
LAST_RESULT = None


def _conv_taps(xf, af, rf, w27_sb, negw27_sb, vecs_sb, nc, ct):
    """Depthwise 3x3x3 stride-2 pad-1 conv for one 128-channel tile.

    The host permutes x's spatial rows into parity-block order: flat index
    (a*4+b*2+c)*1000 + d*100 + h*10 + w  <->  original (2d+a, 2h+b, 2w+c).
    Every tap then reduces to a 2D/3D access pattern. The three taps with
    both dh==-1 and dw==-1 are emitted over the full block with two small
    compensation ops that cancel the row/slice-wrapped reads exactly.

    xf: (p, 8000) fp32 input view; af: (p, 1000) fp32 accumulator;
    rf: (p, 1000) fp32r output (written by the final tap).
    """
    def tap_meta(dd, dh, dw):
        pa, pb, pc = (0 if dd == 0 else 1), (0 if dh == 0 else 1), (0 if dw == 0 else 1)
        Dd, Dh, Dw = (-1 if dd == -1 else 0), (-1 if dh == -1 else 0), (-1 if dw == -1 else 0)
        bb = (pa * 4 + pb * 2 + pc) * 1000
        d0 = 1 if dd == -1 else 0
        t = (dd + 1) * 9 + (dh + 1) * 3 + (dw + 1)
        return bb, Dd, Dh, Dw, d0, t

    def stt(out_ap, in_ap, scal, acc_ap):
        nc.vector.scalar_tensor_tensor(
            out=out_ap, in0=in_ap, scalar=scal, in1=acc_ap,
            op0=Alu.mult, op1=Alu.add)

    def w(t):
        return w27_sb[:, ct, t:t + 1]

    def negw(t):
        return negw27_sb[:, ct, t:t + 1]

    # center tap (0,0,0) + conv bias, full block 0, on the scalar engine
    nc.scalar.activation(af[:, 0:1000], xf[:, 0:1000], Act.Identity,
                         scale=w(13), bias=vecs_sb[:, ct, 1:2])

    taps = [(a, b, c)
            for a in (-1, 0, 1) for b in (-1, 0, 1) for c in (-1, 0, 1)
            if (a, b, c) != (0, 0, 0) and (a, b, c) != (1, 1, 1)]
    for (dd, dh, dw) in taps:
        bb, Dd, Dh, Dw, d0, t = tap_meta(dd, dh, dw)
        dcnt = DR - d0
        i0 = bb + (d0 + Dd) * 100
        ow = af[:, d0 * 100:1000]
        if dh != -1 and dw != -1:
            # case A: contiguous 2D
            i = xf[:, i0:i0 + dcnt * 100]
            stt(ow, i, w(t), ow)
        elif dw == -1 and dh != -1:
            # case B: (d*h merged, w partial) 3D
            o = ow.rearrange("p (x w) -> p x w", w=10)[:, :, 1:10]
            i = xf[:, i0:i0 + dcnt * 100].rearrange("p (x w) -> p x w", w=10)[:, :, 0:9]
            stt(o, i, w(t), o)
        elif dh == -1 and dw != -1:
            # case C: (d, h*w merged) 3D
            o = ow.rearrange("p (d r) -> p d r", r=100)[:, :, 10:100]
            i = xf[:, i0:i0 + dcnt * 100].rearrange("p (d r) -> p d r", r=100)[:, :, 0:90]
            stt(o, i, w(t), o)
        else:
            # case D: dh==-1 and dw==-1 -> extended full-block op + 2 comps
            s = bb + Dd * 100 - 11
            i = xf[:, d0 * 100 + s:1000 + s]
            stt(ow, i, w(t), ow)
            # comp1: out (d, h full, w=0) wrongly read (d, h-1, 9)
            oc1 = ow.rearrange("p (d h w) -> p d h w", h=10, w=10)[:, :, :, 0]
            ic1 = xf[:, d0 * 100 + s:1000 + s].rearrange(
                "p (d h w) -> p d h w", h=10, w=10)[:, :, :, 0]
            stt(oc1, ic1, negw(t), oc1)
            # comp2: out (d, h=0, w 1..9) wrongly read (d-1, 9, w-1)
            oc2 = ow.rearrange("p (d h w) -> p d h w", h=10, w=10)[:, :, 0, 1:10]
            ic2 = xf[:, d0 * 100 + s:1000 + s].rearrange(
                "p (d h w) -> p d h w", h=10, w=10)[:, :, 0, 1:10]
            stt(oc2, ic2, negw(t), oc2)

    # final tap (1,1,1): full block 7, writes the fp32r result
    stt(rf[:, 0:1000], xf[:, 7000:8000], w(26), af[:, 0:1000])


def _ln_over_c(nc, ps, work, ones1_sb, ones128_sb, x_tiles, sq_tiles, width,
               eps_sb=None):
    """Cross-partition LayerNorm stats for C=512 split over 4 partition tiles.

    x_tiles/sq_tiles: lists of 4 fp32r APs, each (128, width).
    Returns (muB, rstdB) PSUM APs (128, width) broadcast down partitions.
    """
    f32, f32r = dt.float32, dt.float32r
    sx = ps.tile([1, width], f32, tag="stat")
    sxx = ps.tile([1, width], f32, tag="stat")
    for ct in range(CT):
        nc.tensor.matmul(sx[:], ones1_sb[:], x_tiles[ct],
                         start=(ct == 0), stop=(ct == CT - 1))
    for ct in range(CT):
        nc.tensor.matmul(sxx[:], ones1_sb[:], sq_tiles[ct],
                         start=(ct == 0), stop=(ct == CT - 1))
    mu_r = work.tile([1, width], f32r, tag="mu")
    nc.vector.tensor_scalar_mul(out=mu_r[:], in0=sx[:], scalar1=1.0 / C)
    msq = work.tile([1, width], f32, tag="msq")
    nc.vector.tensor_scalar_mul(out=msq[:], in0=sxx[:], scalar1=1.0 / C)
    mu2 = work.tile([1, width], f32, tag="mu2")
    nc.vector.tensor_mul(out=mu2[:], in0=mu_r[:].bitcast(f32),
                         in1=mu_r[:].bitcast(f32))
    var = work.tile([1, width], f32, tag="var")
    nc.vector.tensor_sub(out=var[:], in0=msq[:], in1=mu2[:])
    std = work.tile([1, width], f32, tag="std")
    nc.scalar.activation(std[:], var[:], Act.Sqrt, bias=eps_sb[0:1, 0:1])
    rstd_r = work.tile([1, width], f32r, tag="rstd")
    nc.vector.reciprocal(out=rstd_r[:], in_=std[:])
    muB = ps.tile([P, width], f32, tag="bcast")
    nc.tensor.matmul(muB[:], ones128_sb[:], mu_r[:], start=True, stop=True)
    rstdB = ps.tile([P, width], f32, tag="bcast")
    nc.tensor.matmul(rstdB[:], ones128_sb[:], rstd_r[:], start=True, stop=True)
    return muB, rstdB


def _build_program():
    nc = bacc.Bacc("TRN2", target_bir_lowering=False, debug=False, num_devices=8)
    f32, f32r = dt.float32, dt.float32r

    xb = nc.dram_tensor("xb", [N, C], f32, kind="ExternalInput").ap()
    xq = nc.dram_tensor("xq", [NCHUNK, C], f32, kind="ExternalInput").ap()
    wq = nc.dram_tensor("wq", [C, C], f32, kind="ExternalInput").ap()
    wkv = nc.dram_tensor("wkv", [C, 2 * C], f32, kind="ExternalInput").ap()
    wp = nc.dram_tensor("wp", [C, C], f32, kind="ExternalInput").ap()
    w27d = nc.dram_tensor("w27", [C, 27], f32, kind="ExternalInput").ap()
    vecsd = nc.dram_tensor("vecs", [C, 7], f32, kind="ExternalInput").ap()
    bkvd = nc.dram_tensor("bkv", [2 * C], f32, kind="ExternalInput").ap()
    ut = nc.dram_tensor("ut", [NSR, NCHUNK], f32r, kind="ExternalInput").ap()
    eyed = nc.dram_tensor("eye", [P, P], f32, kind="ExternalInput").ap()
    e8d = nc.dram_tensor("e8", [H, C], f32r, kind="ExternalInput").ap()
    ones1d = nc.dram_tensor("ones1", [P, 1], f32r, kind="ExternalInput").ap()
    ones128d = nc.dram_tensor("ones128", [1, P], f32r, kind="ExternalInput").ap()
    epsd = nc.dram_tensor("epsv", [P, 1], f32, kind="ExternalInput").ap()
    yt = nc.dram_tensor("yt", [C, NCHUNK], f32, kind="ExternalOutput").ap()
    # internal DRAM staging for transposed x (channels-major)
    xtd = nc.dram_tensor("xtd", [CT, P, N], f32)

    with TileContext(nc) as tc, nc.allow_low_precision(
            reason="fp32r is fp32-width; rounding feeds fp32r matmuls"):
        with contextlib.ExitStack() as octx:
            consts = octx.enter_context(tc.tile_pool(name="consts", bufs=1))
            keep1 = octx.enter_context(tc.tile_pool(name="keep1", bufs=1))
            work = octx.enter_context(tc.tile_pool(name="work", bufs=2))

            # ---------- constants ----------
            eye_sb = consts.tile([P, P], f32)
            nc.sync.dma_start(out=eye_sb[:], in_=eyed[:])
            e8_sb = consts.tile([H, C], f32r)
            nc.sync.dma_start(out=e8_sb[:], in_=e8d[:])
            ones1_sb = consts.tile([P, 1], f32r)
            nc.sync.dma_start(out=ones1_sb[:], in_=ones1d[:])
            ones128_sb = consts.tile([1, P], f32r)
            nc.sync.dma_start(out=ones128_sb[:], in_=ones128d[:])
            eps_sb = consts.tile([P, 1], f32)
            nc.sync.dma_start(out=eps_sb[:], in_=epsd[:])
            w27_sb = consts.tile([P, CT, 27], f32)
            nc.sync.dma_start(out=w27_sb[:], in_=w27d.rearrange("(o p) t -> p o t", p=P))
            vecs_sb = consts.tile([P, CT, 7], f32)
            nc.sync.dma_start(out=vecs_sb[:], in_=vecsd.rearrange("(o p) t -> p o t", p=P))
            bkv_sb = consts.tile([P, 2 * CT], f32)
            nc.sync.dma_start(out=bkv_sb[:], in_=bkvd.rearrange("(o p) -> p o", p=P))
            negw27_sb = consts.tile([P, CT, 27], f32)
            for ct in range(CT):
                nc.vector.tensor_scalar_mul(out=negw27_sb[:, ct, :],
                                            in0=w27_sb[:, ct, :], scalar1=-1.0)

            qT = keep1.tile([P, CT, NCHUNK], f32r)      # 32 KB/part
            kT = keep1.tile([P, CT, NSR], f32r)         # 16 KB/part

            with contextlib.ExitStack() as ectx:
                ld = ectx.enter_context(tc.tile_pool(name="ld", bufs=2))
                psE = ectx.enter_context(tc.tile_pool(name="psE", bufs=2, space="PSUM"))

                # ---------- Wq -> fp32r ; transpose xq ; q projection ----------
                with tc.tile_pool(name="wqp", bufs=1) as wqp, \
                        tc.tile_pool(name="xqp", bufs=1) as xqp:
                    wq_r = wqp.tile([P, CT, C], f32r)
                    for kt in range(CT):
                        wf = ld.tile([P, C], f32, tag="wld")
                        nc.sync.dma_start(out=wf[:], in_=wq[kt * P:(kt + 1) * P, :])
                        nc.any.tensor_copy(out=wq_r[:, kt, :], in_=wf[:])
                    xqT = xqp.tile([P, CT, NCHUNK], f32r)
                    nq_tiles = (NCHUNK + P - 1) // P
                    for rt in range(nq_tiles):
                        r0 = rt * P
                        rows = min(P, NCHUNK - r0)
                        xa = ld.tile([P, C], f32, tag="xload")
                        nc.sync.dma_start(out=xa[:rows, :], in_=xq[r0:r0 + rows, :])
                        for ct in range(CT):
                            tp = psE.tile([P, P], f32, tag="trps")
                            nc.tensor.transpose(
                                tp[:, :rows], xa[:rows, ct * P:(ct + 1) * P],
                                eye_sb[:rows, :rows])
                            nc.any.tensor_copy(out=xqT[:, ct, r0:r0 + rows],
                                               in_=tp[:, :rows])
                    for ct in range(CT):
                        for rc in range(RC):
                            acc = psE.tile([P, RCW], f32, tag="proj")
                            for kt in range(CT):
                                nc.tensor.matmul(
                                    acc[:], wq_r[:, kt, ct * P:(ct + 1) * P],
                                    xqT[:, kt, rc * RCW:(rc + 1) * RCW],
                                    start=(kt == 0), stop=(kt == CT - 1))
                            nc.any.tensor_scalar_add(
                                out=qT[:, ct, rc * RCW:(rc + 1) * RCW], in0=acc[:],
                                scalar1=vecs_sb[:, ct, 0:1])

                # ---------- transpose xb -> DRAM staging (channels-major) ----------
                nb_tiles = (N + P - 1) // P
                for rt in range(nb_tiles):
                    r0 = rt * P
                    rows = min(P, N - r0)
                    xa = ld.tile([P, C], f32, tag="xload")
                    nc.sync.dma_start(out=xa[:rows, :], in_=xb[r0:r0 + rows, :])
                    for ct in range(CT):
                        tp = psE.tile([P, P], f32, tag="trps")
                        nc.tensor.transpose(
                            tp[:, :rows], xa[:rows, ct * P:(ct + 1) * P],
                            eye_sb[:rows, :rows])
                        sg = ld.tile([P, P], f32, tag="sg")
                        nc.any.tensor_copy(out=sg[:, :rows], in_=tp[:, :rows])
                        nc.sync.dma_start(out=xtd[ct, :, r0:r0 + rows], in_=sg[:, :rows])

                # vT outlives the conv pool (consumed by the v transposes below)
                vpool = ectx.enter_context(tc.tile_pool(name="vpool", bufs=1))
                vT = vpool.tile([P, CT, NSR], f32r)

                with tc.tile_pool(name="cpool", bufs=1) as cpool:
                    # ---------- conv + squares, per channel tile ----------
                    xr_r = cpool.tile([P, CT, NSR], f32r)
                    sq_r = cpool.tile([P, CT, NSR], f32r)
                    for ct in range(CT):
                        xct = cpool.tile([P, N], f32, tag="xct")
                        nc.sync.dma_start(out=xct[:], in_=xtd[ct, :, :])
                        acc_t = cpool.tile([P, NSR], f32, tag="acc")
                        _conv_taps(xct[:], acc_t[:], xr_r[:, ct, :], w27_sb,
                                   negw27_sb, vecs_sb, nc, ct)
                        nc.scalar.activation(sq_r[:, ct, :], xr_r[:, ct, :].bitcast(f32),
                                             Act.Square)

                    # ---------- LayerNorm over C -> xrn_r ----------
                    xrn_r = cpool.tile([P, CT, NSR], f32r)
                    for ch in range(2):
                        cs = slice(ch * RCW, (ch + 1) * RCW)
                        muB, rstdB = _ln_over_c(
                            nc, psE, work, ones1_sb, ones128_sb,
                            [xr_r[:, ct, cs] for ct in range(CT)],
                            [sq_r[:, ct, cs] for ct in range(CT)], RCW, eps_sb)
                        for ct in range(CT):
                            t1 = work.tile([P, RCW], f32, tag="lnt")
                            nc.vector.tensor_sub(out=t1[:], in0=xr_r[:, ct, cs].bitcast(f32),
                                                 in1=muB[:])
                            t2 = work.tile([P, RCW], f32, tag="lnt2")
                            nc.vector.tensor_mul(out=t2[:], in0=t1[:], in1=rstdB[:])
                            nc.vector.tensor_scalar(
                                out=xrn_r[:, ct, cs], in0=t2[:],
                                scalar1=vecs_sb[:, ct, 2:3], scalar2=vecs_sb[:, ct, 3:4],
                                op0=Alu.mult, op1=Alu.add)

                    # ---------- kv projection ----------
                    with tc.tile_pool(name="wkvp", bufs=1) as wkvp:
                        wkv_r = wkvp.tile([P, CT, 2 * C], f32r)
                        for kt in range(CT):
                            for half in range(2):
                                wf = ld.tile([P, C], f32, tag="wld")
                                nc.sync.dma_start(
                                    out=wf[:],
                                    in_=wkv[kt * P:(kt + 1) * P, half * C:(half + 1) * C])
                                nc.any.tensor_copy(
                                    out=wkv_r[:, kt, half * C:(half + 1) * C], in_=wf[:])
                        for mt8 in range(2 * CT):
                            dst = kT[:, mt8, :] if mt8 < CT else vT[:, mt8 - CT, :]
                            for ch in range(2):
                                cs = slice(ch * RCW, (ch + 1) * RCW)
                                acc = psE.tile([P, RCW], f32, tag="proj")
                                for kt in range(CT):
                                    nc.tensor.matmul(
                                        acc[:], wkv_r[:, kt, mt8 * P:(mt8 + 1) * P],
                                        xrn_r[:, kt, cs],
                                        start=(kt == 0), stop=(kt == CT - 1))
                                nc.any.tensor_scalar_add(out=dst[:, cs], in0=acc[:],
                                                         scalar1=bkv_sb[:, mt8:mt8 + 1])

                # ---------- v natural + ones column (v_aug) ----------
                keep2 = octx.enter_context(tc.tile_pool(name="keep2", bufs=1, side="right"))
                v_aug = keep2.tile([P, MT, H, HD + 1], f32r)
                vnp = tc.alloc_tile_pool(name="vnp", bufs=1, side="right")
                v_nat = vnp.tile([P, MT, C], f32r)
                wp_r = keep2.tile([P, CT, C], f32r)
                lnidT = keep2.tile([P, CT, NCHUNK], f32)
                nc.any.tensor_copy(
                    out=v_aug[:, :, :, HD:HD + 1],
                    in_=ones1_sb[:, 0:1, None, None].to_broadcast([P, MT, H, 1]))
                for ci in range(CT):
                    for mt in range(MT):
                        tp = psE.tile([P, P], f32, tag="trps")
                        nc.tensor.transpose(
                            tp[:MTW, :], vT[:, ci, mt * MTW:(mt + 1) * MTW].bitcast(f32),
                            eye_sb[:])
                        nc.any.tensor_copy(
                            out=v_nat[:MTW, mt, ci * P:(ci + 1) * P], in_=tp[:MTW, :])
                        nc.any.tensor_copy(out=v_aug[:MTW, mt, 2 * ci, 0:HD],
                                           in_=tp[:MTW, 0:HD])
                        nc.any.tensor_copy(out=v_aug[:MTW, mt, 2 * ci + 1, 0:HD],
                                           in_=tp[:MTW, HD:2 * HD])
                for kt in range(CT):
                    wf = ld.tile([P, C], f32, tag="wld")
                    nc.sync.dma_start(out=wf[:], in_=wp[kt * P:(kt + 1) * P, :])
                    nc.any.tensor_copy(out=wp_r[:, kt, :], in_=wf[:])

            # ---------- upsampled identity (U matmul) + LayerNorm -> lnidT ----------
            with contextlib.ExitStack() as ictx:
                ld2 = ictx.enter_context(tc.tile_pool(name="ld2", bufs=3))
                psI = ictx.enter_context(tc.tile_pool(name="psI", bufs=2, space="PSUM"))
                idp_pool = ictx.enter_context(tc.tile_pool(name="idp", bufs=2))

                for rc in range(RC):
                    rs = slice(rc * RCW, (rc + 1) * RCW)
                    idr = idp_pool.tile([P, CT, RCW], f32r, tag="idr")
                    idsq = idp_pool.tile([P, CT, RCW], f32r, tag="idsq")
                    ut_t = []
                    for mt in range(MT):
                        u1 = ld2.tile([P, RCW], f32r, tag="uld", bufs=9)
                        nc.sync.dma_start(out=u1[:MTW, :],
                                          in_=ut[mt * MTW:(mt + 1) * MTW, rs])
                        ut_t.append(u1)
                    for ct in range(CT):
                        idp = psI.tile([P, RCW], f32, tag="idps")
                        for mt in range(MT):
                            nc.tensor.matmul(
                                idp[:], v_nat[:MTW, mt, ct * P:(ct + 1) * P],
                                ut_t[mt][:MTW, :],
                                start=(mt == 0), stop=(mt == MT - 1))
                        nc.scalar.activation(idr[:, ct, :], idp[:], Act.Copy)
                        nc.scalar.activation(idsq[:, ct, :], idp[:], Act.Square)
                    muB, rstdB = _ln_over_c(
                        nc, psI, work, ones1_sb, ones128_sb,
                        [idr[:, ct, :] for ct in range(CT)],
                        [idsq[:, ct, :] for ct in range(CT)], RCW, eps_sb)
                    for ct in range(CT):
                        t1 = work.tile([P, RCW], f32, tag="lnt")
                        nc.vector.tensor_sub(out=t1[:], in0=idr[:, ct, :].bitcast(f32),
                                             in1=muB[:])
                        t2 = work.tile([P, RCW], f32, tag="lnt2")
                        nc.vector.tensor_mul(out=t2[:], in0=t1[:], in1=rstdB[:])
                        nc.vector.tensor_scalar(
                            out=lnidT[:, ct, rs], in0=t2[:],
                            scalar1=vecs_sb[:, ct, 4:5], scalar2=vecs_sb[:, ct, 5:6],
                            op0=Alu.mult, op1=Alu.add)

            vnp.release()

            # ---------- attention + normalization + output projection ----------
            with contextlib.ExitStack() as actx:
                psA = actx.enter_context(tc.tile_pool(name="psA", bufs=2, space="PSUM"))
                apool = actx.enter_context(tc.tile_pool(name="apool", bufs=1))
                ppool = actx.enter_context(tc.tile_pool(name="ppool", bufs=2))

                for rc in range(RC):
                    rs = slice(rc * RCW, (rc + 1) * RCW)
                    oT65 = apool.tile([P, H, RCW], f32, tag="ot65")
                    for hh in range(H):
                        pb = HD * (hh % 2)
                        ci = hh // 2
                        pT = ppool.tile([P, MT, RCW], f32r, tag="pt")
                        ov = psA.tile([P, RCW], f32, tag="ovps")
                        for mt in range(MT):
                            sT = psA.tile([P, RCW], f32, tag="scps")
                            nc.tensor.matmul(
                                sT[:MTW, :],
                                kT[pb:pb + HD, ci, mt * MTW:(mt + 1) * MTW],
                                qT[pb:pb + HD, ci, rs],
                                start=True, stop=True)
                            nc.scalar.activation(pT[:MTW, mt, :], sT[:MTW, :], Act.Exp,
                                                 scale=SCALE)
                            nc.tensor.matmul(
                                ov[0:HD + 1, :], v_aug[:MTW, mt, hh, :],
                                pT[:MTW, mt, :],
                                start=(mt == 0), stop=(mt == MT - 1))
                        nc.any.tensor_copy(out=oT65[0:HD + 1, hh, :], in_=ov[0:HD + 1, :])
                    den8 = apool.tile([H, RCW], f32, tag="den8")
                    for hh in range(H):
                        nc.sync.dma_start(out=den8[hh:hh + 1, :],
                                          in_=oT65[HD:HD + 1, hh, :])
                    rec8 = apool.tile([H, RCW], f32r, tag="rec8")
                    nc.vector.reciprocal(out=rec8[:], in_=den8[:])
                    sum_r = apool.tile([P, CT, RCW], f32r, tag="sumr")
                    for ct in range(CT):
                        recB = psA.tile([P, RCW], f32, tag="recB")
                        nc.tensor.matmul(recB[:], e8_sb[:, ct * P:(ct + 1) * P],
                                         rec8[:], start=True, stop=True)
                        tmp = ppool.tile([P, RCW], f32, tag="ntmp")
                        nc.vector.tensor_mul(out=tmp[0:HD, :],
                                             in0=oT65[0:HD, 2 * ct, :],
                                             in1=recB[0:HD, :])
                        nc.vector.tensor_mul(out=tmp[HD:P, :],
                                             in0=oT65[0:HD, 2 * ct + 1, :],
                                             in1=recB[HD:P, :])
                        nc.vector.tensor_add(out=sum_r[:, ct, :], in0=tmp[:],
                                             in1=lnidT[:, ct, rs])
                    for ct2 in range(CT):
                        acc = psA.tile([P, RCW], f32, tag="fin")
                        for kt in range(CT):
                            nc.tensor.matmul(
                                acc[:], wp_r[:, kt, ct2 * P:(ct2 + 1) * P],
                                sum_r[:, kt, :],
                                start=(kt == 0), stop=(kt == CT - 1))
                        oF = ppool.tile([P, RCW], f32, tag="of")
                        nc.any.tensor_scalar_add(out=oF[:], in0=acc[:],
                                                 scalar1=vecs_sb[:, ct2, 6:7])
                        nc.sync.dma_start(out=yt[ct2 * P:(ct2 + 1) * P, rs], in_=oF[:])

    nc.finalize()
    return nc


def _parity_perm():
    perm = np.empty(N, np.int64)
    for a in range(2):
        for b in range(2):
            for c in range(2):
                blk = (a * 4 + b * 2 + c) * NSR
                for d in range(DR):
                    for h in range(DR):
                        for w_ in range(DR):
                            perm[blk + d * 100 + h * 10 + w_] = (
                                (2 * d + a) * 400 + (2 * h + b) * 20 + (2 * w_ + c))
    return perm


def _host_consts():
    eye = np.eye(P, dtype=np.float32)
    e8 = np.zeros((H, C), np.float32)
    for p in range(C):
        hh = 2 * (p // P) + (p % P) // HD
        e8[hh, p] = 1.0
    ones1 = np.ones((P, 1), np.float32)
    ones128 = np.ones((1, P), np.float32)
    epsv = np.full((P, 1), EPS, np.float32)
    return eye, e8, ones1, ones128, epsv


def _interp_1d(n_out, n_in, off):
    out = []
    for i in range(n_out):
        src = (off + i + 0.5) / 2.0 - 0.5
        lo = int(np.floor(src))
        f = src - lo
        lo_c = min(max(lo, 0), n_in - 1)
        hi_c = min(max(lo + 1, 0), n_in - 1)
        out.append(((lo_c, 1.0 - f), (hi_c, f)))
    return out


def _build_ut(j):
    """U^T (NSR, NCHUNK): idT[:, n] = sum_m v_nat[m, :] * UT[m, n], quarter j."""
    ut = np.zeros((NSR, NCHUNK), np.float32)
    d_lo = (j * NCHUNK) // (D3 * D3)
    dmap = _interp_1d(5, DR, d_lo)
    hmap = _interp_1d(D3, DR, 0)
    wmap = _interp_1d(D3, DR, 0)
    for dd in range(5):
        for hh2 in range(D3):
            for ww in range(D3):
                nloc = dd * D3 * D3 + hh2 * D3 + ww
                for (di, dwt) in dmap[dd]:
                    for (hi, hwt) in hmap[hh2]:
                        for (wi, wwt) in wmap[ww]:
                            m = di * DR * DR + hi * DR + wi
                            ut[m, nloc] += dwt * hwt * wwt
    return ut


def kernel(**inputs):
    global _PROGRAM
    x = np.asarray(inputs["x"], np.float32)
    Wq = np.ascontiguousarray(np.asarray(inputs["Wq"], np.float32))
    bq = np.asarray(inputs["bq"], np.float32)
    Wkv = np.ascontiguousarray(np.asarray(inputs["Wkv"], np.float32))
    bkv_ = np.asarray(inputs["bkv"], np.float32)
    sr_w = np.asarray(inputs["sr_w"], np.float32)
    sr_b = np.asarray(inputs["sr_b"], np.float32)
    sr_g = np.asarray(inputs["sr_g"], np.float32)
    sr_beta = np.asarray(inputs["sr_beta"], np.float32)
    up_g = np.asarray(inputs["up_g"], np.float32)
    up_beta = np.asarray(inputs["up_beta"], np.float32)
    Wp = np.ascontiguousarray(np.asarray(inputs["Wp"], np.float32))
    bp = np.asarray(inputs["bp"], np.float32)

    if _PROGRAM is None:
        _PROGRAM = _build_program()
    nc = _PROGRAM

    eye, e8, ones1, ones128, epsv = _host_consts()
    w27 = np.ascontiguousarray(sr_w.reshape(C, 27))
    vecs = np.ascontiguousarray(
        np.stack([bq, sr_b, sr_g, sr_beta, up_g, up_beta, bp], axis=1))
    uts = [_build_ut(j) for j in range(4)]
    perm = _parity_perm()

    in_maps = []
    for core in range(8):
        b, j = core // 4, core % 4
        in_maps.append({
            "xb": np.ascontiguousarray(x[b][perm]),
            "xq": np.ascontiguousarray(x[b, j * NCHUNK:(j + 1) * NCHUNK]),
            "wq": Wq, "wkv": Wkv, "wp": Wp,
            "w27": w27, "vecs": vecs, "bkv": bkv_,
            "ut": uts[j],
            "eye": eye, "e8": e8, "ones1": ones1, "ones128": ones128,
            "epsv": epsv,
        })

    global LAST_RESULT
    res = run_bass_kernel_spmd(nc, in_maps, core_ids=list(range(8)), trace=TRACE)
    LAST_RESULT = res
    out = np.empty((B, N, C), np.float32)
    for core in range(8):
        b, j = core // 4, core % 4
        out[b, j * NCHUNK:(j + 1) * NCHUNK, :] = res.results[core]["yt"].T
    return out



# revision 7
# speedup vs baseline: 2.0581x; 2.0581x over previous
"""Sparse-attention (PVT-style SRA) kernel for 8 Trainium2 NeuronCores.

Sharding: 8 cores = 2 batches x 4 row-quarters of N=8000. Each core computes
its 2000 output rows end-to-end; the spatial-reduction branch (conv+LN+kv) is
replicated per batch. All matmuls run in fp16 (1 cycle/row on the PE, fast
weight load); accumulation stays fp32 in PSUM. The depthwise conv runs on the
vector engine in fp16 with per-partition tap scalars; the trilinear upsample
is an interpolation matmul whose weights (products of 1/4 and 3/4) are
fp16-exact. Host pre-transposes x into channel-major layout so the kernel
does no PE transposes of the input.
"""

import sys

sys.path.insert(0, "/opt/trn_rl_repo")

import contextlib
import numpy as np
import concourse.bacc as bacc
import concourse.mybir as mybir
from concourse.tile import TileContext
from concourse.bass_utils import run_bass_kernel_spmd

dt = mybir.dt
Alu = mybir.AluOpType
Act = mybir.ActivationFunctionType

P = 128
B, N, C = 2, 8000, 512
H, HD = 8, 64
D3 = 20          # full spatial edge (D=H=W)
DR = 10          # reduced spatial edge
NSR = 1000       # DR**3
CT = 4           # C // P
NCHUNK = 2000    # output rows per core
RC = 4           # row chunks per core
RCW = 500        # rows per chunk
MT = 8           # key tiles
MTW = 125        # keys per tile
BW = 512         # PSUM bank width (f32 elems)
SCALE = HD ** -0.5
EPS = 1e-6

_PROGRAM = None
_HOST = None
TRACE = False
LAST_RESULT = None


def _conv_taps(xf, af, rf, w27_sb, negw27_sb, vecs_sb, nc, ct):
    """Depthwise 3x3x3 stride-2 pad-1 conv for one 128-channel tile (fp16).

    The host permutes x's spatial rows into parity-block order: flat index
    (a*4+b*2+c)*1000 + d*100 + h*10 + w  <->  original (2d+a, 2h+b, 2w+c).
    Every tap then reduces to a 2D/3D access pattern. The three taps with
    both dh==-1 and dw==-1 are emitted over the full block with two small
    compensation ops that cancel the row/slice-wrapped reads exactly.

    xf: (p, 8000) fp16 input view; af: (p, 1000) fp16 accumulator;
    rf: (p, 1000) fp16 output (written by the final tap).
    """
    def tap_meta(dd, dh, dw):
        pa, pb, pc = (0 if dd == 0 else 1), (0 if dh == 0 else 1), (0 if dw == 0 else 1)
        Dd, Dh, Dw = (-1 if dd == -1 else 0), (-1 if dh == -1 else 0), (-1 if dw == -1 else 0)
        bb = (pa * 4 + pb * 2 + pc) * 1000
        d0 = 1 if dd == -1 else 0
        t = (dd + 1) * 9 + (dh + 1) * 3 + (dw + 1)
        return bb, Dd, Dh, Dw, d0, t

    def stt(out_ap, in_ap, scal, acc_ap):
        nc.vector.scalar_tensor_tensor(
            out=out_ap, in0=in_ap, scalar=scal, in1=acc_ap,
            op0=Alu.mult, op1=Alu.add)

    def w(t):
        return w27_sb[:, ct, t:t + 1]

    def negw(t):
        return negw27_sb[:, ct, t:t + 1]

    # center tap (0,0,0) + conv bias, full block 0, on the scalar engine
    nc.scalar.activation(af[:, 0:1000], xf[:, 0:1000], Act.Identity,
                         scale=w(13), bias=vecs_sb[:, ct, 1:2])

    taps = [(a, b, c)
            for a in (-1, 0, 1) for b in (-1, 0, 1) for c in (-1, 0, 1)
            if (a, b, c) != (0, 0, 0) and (a, b, c) != (1, 1, 1)]
    for (dd, dh, dw) in taps:
        bb, Dd, Dh, Dw, d0, t = tap_meta(dd, dh, dw)
        dcnt = DR - d0
        i0 = bb + (d0 + Dd) * 100
        ow = af[:, d0 * 100:1000]
        if dh != -1 and dw != -1:
            # case A: contiguous 2D
            i = xf[:, i0:i0 + dcnt * 100]
            stt(ow, i, w(t), ow)
        elif dw == -1 and dh != -1:
            # case B: (d*h merged, w partial) 3D
            o = ow.rearrange("p (x w) -> p x w", w=10)[:, :, 1:10]
            i = xf[:, i0:i0 + dcnt * 100].rearrange("p (x w) -> p x w", w=10)[:, :, 0:9]
            stt(o, i, w(t), o)
        elif dh == -1 and dw != -1:
            # case C: (d, h*w merged) 3D
            o = ow.rearrange("p (d r) -> p d r", r=100)[:, :, 10:100]
            i = xf[:, i0:i0 + dcnt * 100].rearrange("p (d r) -> p d r", r=100)[:, :, 0:90]
            stt(o, i, w(t), o)
        else:
            # case D: dh==-1 and dw==-1 -> extended full-block op + 2 comps
            s = bb + Dd * 100 - 11
            i = xf[:, d0 * 100 + s:1000 + s]
            stt(ow, i, w(t), ow)
            # comp1: out (d, h full, w=0) wrongly read (d, h-1, 9)
            oc1 = ow.rearrange("p (d h w) -> p d h w", h=10, w=10)[:, :, :, 0]
            ic1 = xf[:, d0 * 100 + s:1000 + s].rearrange(
                "p (d h w) -> p d h w", h=10, w=10)[:, :, :, 0]
            stt(oc1, ic1, negw(t), oc1)
            # comp2: out (d, h=0, w 1..9) wrongly read (d-1, 9, w-1)
            oc2 = ow.rearrange("p (d h w) -> p d h w", h=10, w=10)[:, :, 0, 1:10]
            ic2 = xf[:, d0 * 100 + s:1000 + s].rearrange(
                "p (d h w) -> p d h w", h=10, w=10)[:, :, 0, 1:10]
            stt(oc2, ic2, negw(t), oc2)

    # final tap (1,1,1): full block 7, writes the result
    stt(rf[:, 0:1000], xf[:, 7000:8000], w(26), af[:, 0:1000])


def _ln_stats(nc, work, ones1_sb, ones128_sb, x_tiles, sq_tiles, width,
              eps_sb, sx, sxx, muB, rstdB):
    """Cross-partition LayerNorm stats for C=512 split over 4 partition tiles.

    x_tiles/sq_tiles: lists of 4 fp16 APs, each (128, width). sx/sxx are
    (1, width) f32 PSUM APs; muB/rstdB are (128, width) f32 PSUM APs that
    receive the broadcast mean / inverse-std.
    """
    f32, f16 = dt.float32, dt.float16
    n = len(x_tiles)
    for i, xt in enumerate(x_tiles):
        nc.tensor.matmul(sx, ones1_sb[:], xt, start=(i == 0), stop=(i == n - 1))
    for i, st in enumerate(sq_tiles):
        nc.tensor.matmul(sxx, ones1_sb[:], st, start=(i == 0), stop=(i == n - 1))
    mu_r = work.tile([1, width], f16, tag="mu", bufs=2)
    nc.vector.tensor_scalar_mul(out=mu_r[:], in0=sx, scalar1=1.0 / C)
    msq = work.tile([1, width], f32, tag="msq", bufs=2)
    nc.vector.tensor_scalar_mul(out=msq[:], in0=sxx, scalar1=1.0 / C)
    mu2 = work.tile([1, width], f32, tag="mu2", bufs=2)
    nc.vector.tensor_mul(out=mu2[:], in0=mu_r[:], in1=mu_r[:])
    var = work.tile([1, width], f32, tag="var", bufs=2)
    nc.vector.tensor_sub(out=var[:], in0=msq[:], in1=mu2[:])
    std = work.tile([1, width], f32, tag="std", bufs=2)
    nc.scalar.activation(std[:], var[:], Act.Sqrt, bias=eps_sb[0:1, 0:1])
    rstd_r = work.tile([1, width], f16, tag="rstd", bufs=2)
    nc.vector.reciprocal(out=rstd_r[:], in_=std[:])
    nc.tensor.matmul(muB, ones128_sb[:], mu_r[:], start=True, stop=True)
    nc.tensor.matmul(rstdB, ones128_sb[:], rstd_r[:], start=True, stop=True)


def _build_program():
    nc = bacc.Bacc("TRN2", target_bir_lowering=False, debug=False, num_devices=8)
    f32, f16 = dt.float32, dt.float16

    xqtd = nc.dram_tensor("xqt", [CT, P, NCHUNK], f16, kind="ExternalInput").ap()
    xtd = nc.dram_tensor("xtd", [CT, P, N], f16, kind="ExternalInput").ap()
    wq = nc.dram_tensor("wq", [C, C], f16, kind="ExternalInput").ap()
    wkv = nc.dram_tensor("wkv", [C, 2 * C], f16, kind="ExternalInput").ap()
    wp = nc.dram_tensor("wp", [C, C], f16, kind="ExternalInput").ap()
    w27d = nc.dram_tensor("w27", [C, 27], f32, kind="ExternalInput").ap()
    vecsd = nc.dram_tensor("vecs", [C, 7], f32, kind="ExternalInput").ap()
    bkvd = nc.dram_tensor("bkv", [2 * C], f32, kind="ExternalInput").ap()
    utd = nc.dram_tensor("ut", [NSR, NCHUNK], f16, kind="ExternalInput").ap()
    eyed = nc.dram_tensor("eye", [P, P], f16, kind="ExternalInput").ap()
    e8d = nc.dram_tensor("e8", [H, C], f16, kind="ExternalInput").ap()
    ones1d = nc.dram_tensor("ones1", [P, 1], f16, kind="ExternalInput").ap()
    ones128d = nc.dram_tensor("ones128", [1, P], f16, kind="ExternalInput").ap()
    epsd = nc.dram_tensor("epsv", [P, 1], f32, kind="ExternalInput").ap()
    yt = nc.dram_tensor("yt", [C, NCHUNK], f16, kind="ExternalOutput").ap()

    with TileContext(nc) as tc, nc.allow_low_precision(
            reason="fp16 data with fp32 PSUM accumulation; tol is 2e-2"):
        with contextlib.ExitStack() as octx:
            consts = octx.enter_context(tc.tile_pool(name="consts", bufs=1))
            keep1 = octx.enter_context(tc.tile_pool(name="keep1", bufs=1))
            work = octx.enter_context(tc.tile_pool(name="work", bufs=2))

            # ---------- constants ----------
            eye_sb = consts.tile([P, P], f16)
            nc.sync.dma_start(out=eye_sb[:], in_=eyed[:])
            e8_sb = consts.tile([H, C], f16)
            nc.sync.dma_start(out=e8_sb[:], in_=e8d[:])
            ones1_sb = consts.tile([P, 1], f16)
            nc.sync.dma_start(out=ones1_sb[:], in_=ones1d[:])
            ones128_sb = consts.tile([1, P], f16)
            nc.sync.dma_start(out=ones128_sb[:], in_=ones128d[:])
            eps_sb = consts.tile([P, 1], f32)
            nc.sync.dma_start(out=eps_sb[:], in_=epsd[:])
            w27_sb = consts.tile([P, CT, 27], f32)
            nc.sync.dma_start(out=w27_sb[:], in_=w27d.rearrange("(o p) t -> p o t", p=P))
            vecs_sb = consts.tile([P, CT, 7], f32)
            nc.sync.dma_start(out=vecs_sb[:], in_=vecsd.rearrange("(o p) t -> p o t", p=P))
            bkv_sb = consts.tile([P, 2 * CT], f32)
            nc.sync.dma_start(out=bkv_sb[:], in_=bkvd.rearrange("(o p) -> p o", p=P))
            negw27_sb = consts.tile([P, CT, 27], f32)
            for ct in range(CT):
                nc.vector.tensor_scalar_mul(out=negw27_sb[:, ct, :],
                                            in0=w27_sb[:, ct, :], scalar1=-1.0)

            qT = keep1.tile([P, CT, NCHUNK], f16)       # 16 KB/part
            kT = keep1.tile([P, CT, NSR], f16)          # 8 KB/part
            lnidT = keep1.tile([P, CT, NCHUNK], f16)    # 16 KB/part
            wp_sb = keep1.tile([P, CT, C], f16)
            v_nat = keep1.tile([P, MT, C], f16)
            v_aug = keep1.tile([P, MT, H, HD + 1], f16)

            with contextlib.ExitStack() as ectx:
                psE = ectx.enter_context(tc.tile_pool(name="psE", bufs=2, space="PSUM"))

                with tc.tile_pool(name="cpool", bufs=1) as cpool, \
                        tc.tile_pool(name="wqp", bufs=1) as wqp:
                    # ---------- conv input + weight/xq loads ----------
                    wq_sb = wqp.tile([P, CT, C], f16)
                    nc.sync.dma_start(out=wq_sb[:],
                                      in_=wq.rearrange("(k p) m -> p k m", p=P))
                    xqT = wqp.tile([P, CT, NCHUNK], f16)
                    for ct in range(CT):
                        nc.sync.dma_start(out=xqT[:, ct, :], in_=xqtd[ct, :, :])
                    wkv_sb = wqp.tile([P, CT, 2 * C], f16)
                    nc.sync.dma_start(out=wkv_sb[:],
                                      in_=wkv.rearrange("(k p) m -> p k m", p=P))
                    nc.sync.dma_start(out=wp_sb[:],
                                      in_=wp.rearrange("(k p) m -> p k m", p=P))

                    # ---------- conv + squares, per channel tile (DVE) ----------
                    xr = cpool.tile([P, CT, NSR], f16)
                    sq = cpool.tile([P, CT, NSR], f16)
                    for ct in range(CT):
                        xct = cpool.tile([P, N], f16, tag="xct", bufs=2)
                        nc.sync.dma_start(out=xct[:], in_=xtd[ct, :, :])
                        acc_t = cpool.tile([P, NSR], f16, tag="acc", bufs=2)
                        _conv_taps(xct[:], acc_t[:], xr[:, ct, :], w27_sb,
                                   negw27_sb, vecs_sb, nc, ct)
                        nc.scalar.activation(sq[:, ct, :], xr[:, ct, :], Act.Square)

                    # ---------- q projection (PE, overlaps conv) ----------
                    for ct in range(CT):
                        for rc in range(RC):
                            acc = psE.tile([P, RCW], f32, tag="proj")
                            for kt in range(CT):
                                nc.tensor.matmul(
                                    acc[:], wq_sb[:, kt, ct * P:(ct + 1) * P],
                                    xqT[:, kt, rc * RCW:(rc + 1) * RCW],
                                    start=(kt == 0), stop=(kt == CT - 1))
                            nc.scalar.activation(
                                qT[:, ct, rc * RCW:(rc + 1) * RCW], acc[:],
                                Act.Identity, bias=vecs_sb[:, ct, 0:1])

                    # ---------- LayerNorm over C -> xrn ----------
                    xrn = cpool.tile([P, CT, NSR], f16)
                    for ch in range(2):
                        cs = slice(ch * RCW, (ch + 1) * RCW)
                        sx = psE.tile([1, RCW], f32, tag="stat")
                        sxx = psE.tile([1, RCW], f32, tag="stat")
                        muB = psE.tile([P, RCW], f32, tag="bcast")
                        rstdB = psE.tile([P, RCW], f32, tag="bcast")
                        _ln_stats(nc, work, ones1_sb, ones128_sb,
                                  [xr[:, ct, cs] for ct in range(CT)],
                                  [sq[:, ct, cs] for ct in range(CT)], RCW,
                                  eps_sb, sx[:], sxx[:], muB[:], rstdB[:])
                        for ct in range(CT):
                            t1 = work.tile([P, RCW], f32, tag="lnt")
                            nc.vector.tensor_sub(out=t1[:], in0=xr[:, ct, cs],
                                                 in1=muB[:])
                            t2 = work.tile([P, RCW], f32, tag="lnt2")
                            nc.vector.tensor_mul(out=t2[:], in0=t1[:], in1=rstdB[:])
                            nc.vector.tensor_scalar(
                                out=xrn[:, ct, cs], in0=t2[:],
                                scalar1=vecs_sb[:, ct, 2:3], scalar2=vecs_sb[:, ct, 3:4],
                                op0=Alu.mult, op1=Alu.add)

                    # ---------- kv projection ----------
                    vT = cpool.tile([P, CT, NSR], f16)
                    for mt8 in range(2 * CT):
                        dsts = kT if mt8 < CT else vT
                        di = mt8 if mt8 < CT else mt8 - CT
                        for ch in range(2):
                            cs = slice(ch * RCW, (ch + 1) * RCW)
                            acc = psE.tile([P, RCW], f32, tag="proj")
                            for kt in range(CT):
                                nc.tensor.matmul(
                                    acc[:], wkv_sb[:, kt, mt8 * P:(mt8 + 1) * P],
                                    xrn[:, kt, cs],
                                    start=(kt == 0), stop=(kt == CT - 1))
                            nc.vector.tensor_scalar_add(
                                out=dsts[:, di, cs], in0=acc[:],
                                scalar1=bkv_sb[:, mt8:mt8 + 1])

                    # ---------- v natural + ones column (v_aug) ----------
                    nc.gpsimd.tensor_copy(
                        out=v_aug[:, :, :, HD:HD + 1],
                        in_=ones1_sb[:, 0:1, None, None].to_broadcast([P, MT, H, 1]))
                    for ci in range(CT):
                        for mt in range(MT):
                            tp = psE.tile([P, P], f16, tag="trps")
                            nc.tensor.transpose(
                                tp[:MTW, :], vT[:, ci, mt * MTW:(mt + 1) * MTW],
                                eye_sb[:])
                            nc.vector.tensor_copy(
                                out=v_nat[:MTW, mt, ci * P:(ci + 1) * P],
                                in_=tp[:MTW, :])
                            nc.vector.tensor_copy(out=v_aug[:MTW, mt, 2 * ci, 0:HD],
                                                  in_=tp[:MTW, 0:HD])
                            nc.vector.tensor_copy(out=v_aug[:MTW, mt, 2 * ci + 1, 0:HD],
                                                  in_=tp[:MTW, HD:2 * HD])

            # ---------- per row-chunk: identity (U matmul) + LN + attention ----
            with contextlib.ExitStack() as actx:
                psA = actx.enter_context(tc.tile_pool(name="psA", bufs=1, space="PSUM"))
                ld2 = actx.enter_context(tc.tile_pool(name="ld2", bufs=2))
                ppool = actx.enter_context(tc.tile_pool(name="ppool", bufs=2))
                apool = actx.enter_context(tc.tile_pool(name="apool", bufs=1))

                for rc in range(RC):
                    rs = slice(rc * RCW, (rc + 1) * RCW)

                    # --- identity branch: idT = v_nat^T @ U^T, then LN ---
                    ut_t = []
                    for mt in range(MT):
                        u1 = ld2.tile([P, RCW], f16, tag="uld", bufs=10)
                        nc.sync.dma_start(out=u1[:MTW, :],
                                          in_=utd[mt * MTW:(mt + 1) * MTW, rs])
                        ut_t.append(u1)
                    idr = apool.tile([P, CT, RCW], f16, tag="idr", bufs=2)
                    idsq = apool.tile([P, CT, RCW], f16, tag="idsq", bufs=2)
                    for cp in range(2):          # ct pairs share a 2-bank tile
                        idp = psA.tile([P, 2, BW], f32, tag="sc", bufs=3)
                        for k in range(2):
                            ct = 2 * cp + k
                            for mt in range(MT):
                                nc.tensor.matmul(
                                    idp[:, k, 0:RCW],
                                    v_nat[:MTW, mt, ct * P:(ct + 1) * P],
                                    ut_t[mt][:MTW, :],
                                    start=(mt == 0), stop=(mt == MT - 1))
                        nc.vector.tensor_copy(out=idr[:, 2 * cp:2 * cp + 2, :],
                                              in_=idp[:, :, 0:RCW])
                        nc.scalar.activation(idsq[:, 2 * cp:2 * cp + 2, :],
                                             idp[:, :, 0:RCW], Act.Square)
                    sxt = psA.tile([P, BW], f32, tag="ov", bufs=2)
                    sxxt = psA.tile([P, BW], f32, tag="ov", bufs=2)
                    mb = psA.tile([P, 2, BW], f32, tag="sc", bufs=3)
                    _ln_stats(nc, work, ones1_sb, ones128_sb,
                              [idr[:, ct, :] for ct in range(CT)],
                              [idsq[:, ct, :] for ct in range(CT)], RCW,
                              eps_sb, sxt[0:1, 0:RCW], sxxt[0:1, 0:RCW],
                              mb[:, 0, 0:RCW], mb[:, 1, 0:RCW])
                    for ct in range(CT):
                        t1 = work.tile([P, RCW], f32, tag="lnt")
                        nc.vector.tensor_sub(out=t1[:], in0=idr[:, ct, :],
                                             in1=mb[:, 0, 0:RCW])
                        t2 = work.tile([P, RCW], f32, tag="lnt2")
                        nc.vector.tensor_mul(out=t2[:], in0=t1[:], in1=mb[:, 1, 0:RCW])
                        nc.vector.tensor_scalar(
                            out=lnidT[:, ct, rs], in0=t2[:],
                            scalar1=vecs_sb[:, ct, 4:5], scalar2=vecs_sb[:, ct, 5:6],
                            op0=Alu.mult, op1=Alu.add)

                    # --- attention ---
                    oT65 = apool.tile([P, H, RCW], f16, tag="ot65", bufs=2)
                    for hh in range(H):
                        pb = HD * (hh % 2)
                        ci = hh // 2
                        pT = ppool.tile([P, MT, BW], f16, tag="pt")
                        ov = psA.tile([P, BW], f32, tag="ov", bufs=2)
                        for g in range(4):
                            sc = psA.tile([P, 2, BW], f32, tag="sc", bufs=3)
                            for k in range(2):
                                mt = 2 * g + k
                                nc.tensor.matmul(
                                    sc[:MTW, k, 0:RCW],
                                    kT[pb:pb + HD, ci, mt * MTW:(mt + 1) * MTW],
                                    qT[pb:pb + HD, ci, rs],
                                    start=True, stop=True)
                            nc.scalar.activation(pT[:MTW, 2 * g:2 * g + 2, :],
                                                 sc[:MTW, :, :], Act.Exp,
                                                 scale=SCALE)
                        for mt in range(MT):
                            nc.tensor.matmul(
                                ov[0:HD + 1, 0:RCW], v_aug[:MTW, mt, hh, :],
                                pT[:MTW, mt, 0:RCW],
                                start=(mt == 0), stop=(mt == MT - 1))
                        nc.vector.tensor_copy(out=oT65[0:HD + 1, hh, :],
                                              in_=ov[0:HD + 1, 0:RCW])

                    # --- normalize + add identity + output projection ---
                    den8 = apool.tile([H, RCW], f16, tag="den8", bufs=2)
                    for hh in range(H):
                        nc.sync.dma_start(out=den8[hh:hh + 1, :],
                                          in_=oT65[HD:HD + 1, hh, :])
                    rec8 = apool.tile([H, RCW], f16, tag="rec8", bufs=2)
                    nc.vector.reciprocal(out=rec8[:], in_=den8[:])
                    sum_r = apool.tile([P, CT, RCW], f16, tag="sumr", bufs=2)
                    for ct in range(CT):
                        recB = psA.tile([P, 2, BW], f32, tag="sc", bufs=3)
                        nc.tensor.matmul(recB[:, 0, 0:RCW],
                                         e8_sb[:, ct * P:(ct + 1) * P],
                                         rec8[:], start=True, stop=True)
                        tmp = work.tile([P, RCW], f32, tag="ntmp")
                        nc.vector.tensor_mul(out=tmp[0:HD, :],
                                             in0=oT65[0:HD, 2 * ct, :],
                                             in1=recB[0:HD, 0, 0:RCW])
                        nc.vector.tensor_mul(out=tmp[HD:P, :],
                                             in0=oT65[0:HD, 2 * ct + 1, :],
                                             in1=recB[HD:P, 0, 0:RCW])
                        nc.vector.tensor_add(out=sum_r[:, ct, :], in0=tmp[:],
                                             in1=lnidT[:, ct, rs])
                    for ct2 in range(CT):
                        fin = psA.tile([P, 2, BW], f32, tag="sc", bufs=3)
                        for kt in range(CT):
                            nc.tensor.matmul(
                                fin[:, 0, 0:RCW],
                                wp_sb[:, kt, ct2 * P:(ct2 + 1) * P],
                                sum_r[:, kt, :],
                                start=(kt == 0), stop=(kt == CT - 1))
                        oF = apool.tile([P, RCW], f16, tag="of", bufs=2)
                        nc.vector.tensor_scalar_add(out=oF[:], in0=fin[:, 0, 0:RCW],
                                                    scalar1=vecs_sb[:, ct2, 6:7])
                        nc.sync.dma_start(out=yt[ct2 * P:(ct2 + 1) * P, rs], in_=oF[:])

    nc.finalize()
    return nc


def _parity_perm():
    perm = np.empty(N, np.int64)
    for a in range(2):
        for b in range(2):
            for c in range(2):
                blk = (a * 4 + b * 2 + c) * NSR
                for d in range(DR):
                    for h in range(DR):
                        for w_ in range(DR):
                            perm[blk + d * 100 + h * 10 + w_] = (
                                (2 * d + a) * 400 + (2 * h + b) * 20 + (2 * w_ + c))
    return perm


def _host_consts():
    eye = np.eye(P, dtype=np.float16)
    e8 = np.zeros((H, C), np.float16)
    for p in range(C):
        hh = 2 * (p // P) + (p % P) // HD
        e8[hh, p] = 1.0
    ones1 = np.ones((P, 1), np.float16)
    ones128 = np.ones((1, P), np.float16)
    epsv = np.full((P, 1), EPS, np.float32)
    return eye, e8, ones1, ones128, epsv


def _interp_1d(n_out, n_in, off):
    out = []
    for i in range(n_out):
        src = (off + i + 0.5) / 2.0 - 0.5
        lo = int(np.floor(src))
        f = src - lo
        lo_c = min(max(lo, 0), n_in - 1)
        hi_c = min(max(lo + 1, 0), n_in - 1)
        out.append(((lo_c, 1.0 - f), (hi_c, f)))
    return out


def _build_ut(j):
    """U^T (NSR, NCHUNK): idT[:, n] = sum_m v_nat[m, :] * UT[m, n], quarter j."""
    ut = np.zeros((NSR, NCHUNK), np.float32)
    d_lo = (j * NCHUNK) // (D3 * D3)
    dmap = _interp_1d(5, DR, d_lo)
    hmap = _interp_1d(D3, DR, 0)
    wmap = _interp_1d(D3, DR, 0)
    for dd in range(5):
        for hh2 in range(D3):
            for ww in range(D3):
                nloc = dd * D3 * D3 + hh2 * D3 + ww
                for (di, dwt) in dmap[dd]:
                    for (hi, hwt) in hmap[hh2]:
                        for (wi, wwt) in wmap[ww]:
                            m = di * DR * DR + hi * DR + wi
                            ut[m, nloc] += dwt * hwt * wwt
    return ut.astype(np.float16)


def kernel(**inputs):
    global _PROGRAM, _HOST, LAST_RESULT
    x = np.asarray(inputs["x"], np.float32)
    Wq = np.asarray(inputs["Wq"], np.float32)
    bq = np.asarray(inputs["bq"], np.float32)
    Wkv = np.asarray(inputs["Wkv"], np.float32)
    bkv_ = np.asarray(inputs["bkv"], np.float32)
    sr_w = np.asarray(inputs["sr_w"], np.float32)
    sr_b = np.asarray(inputs["sr_b"], np.float32)
    sr_g = np.asarray(inputs["sr_g"], np.float32)
    sr_beta = np.asarray(inputs["sr_beta"], np.float32)
    up_g = np.asarray(inputs["up_g"], np.float32)
    up_beta = np.asarray(inputs["up_beta"], np.float32)
    Wp = np.asarray(inputs["Wp"], np.float32)
    bp = np.asarray(inputs["bp"], np.float32)

    if _PROGRAM is None:
        _PROGRAM = _build_program()
    nc = _PROGRAM

    if _HOST is None:
        _HOST = (_host_consts(), [_build_ut(j) for j in range(4)], _parity_perm())
    (eye, e8, ones1, ones128, epsv), uts, perm = _HOST

    w27 = np.ascontiguousarray(sr_w.reshape(C, 27))
    vecs = np.ascontiguousarray(
        np.stack([bq, sr_b, sr_g, sr_beta, up_g, up_beta, bp], axis=1))
    wq16 = np.ascontiguousarray(Wq.astype(np.float16))
    wkv16 = np.ascontiguousarray(Wkv.astype(np.float16))
    wp16 = np.ascontiguousarray(Wp.astype(np.float16))

    xtds, xqts = [], []
    for b in range(B):
        xtds.append(np.ascontiguousarray(
            x[b][perm].T.reshape(CT, P, N).astype(np.float16)))
        xqts.append([np.ascontiguousarray(
            x[b, j * NCHUNK:(j + 1) * NCHUNK].T.reshape(CT, P, NCHUNK)
            .astype(np.float16)) for j in range(4)])

    in_maps = []
    for core in range(8):
        b, j = core // 4, core % 4
        in_maps.append({
            "xtd": xtds[b],
            "xqt": xqts[b][j],
            "wq": wq16, "wkv": wkv16, "wp": wp16,
            "w27": w27, "vecs": vecs, "bkv": bkv_,
            "ut": uts[j],
            "eye": eye, "e8": e8, "ones1": ones1, "ones128": ones128,
            "epsv": epsv,
        })

    res = run_bass_kernel_spmd(nc, in_maps, core_ids=list(range(8)), trace=TRACE)
    LAST_RESULT = res
    out = np.empty((B, N, C), np.float32)
    for core in range(8):
        b, j = core // 4, core % 4
        out[b, j * NCHUNK:(j + 1) * NCHUNK, :] = (
            res.results[core]["yt"].astype(np.float32).T)
    return out


# revision 8
# speedup vs baseline: 2.1433x; 1.0414x over previous
"""Sparse-attention (PVT-style SRA) kernel for 8 Trainium2 NeuronCores.

Sharding: 8 cores = 2 batches x 4 row-quarters of N=8000. Each core computes
its 2000 output rows end-to-end; the spatial-reduction branch (conv+LN+kv) is
replicated per batch. All matmuls run in fp16 (1 cycle/row on the PE, fast
weight load); accumulation stays fp32 in PSUM. The depthwise conv runs on the
vector engine in fp16 with per-partition tap scalars; the trilinear upsample
is an interpolation matmul whose weights (products of 1/4 and 3/4) are
fp16-exact. Host pre-transposes x into channel-major layout so the kernel
does no PE transposes of the input.
"""

import sys

sys.path.insert(0, "/opt/trn_rl_repo")

import contextlib
import numpy as np
import ml_dtypes
import concourse.bacc as bacc
import concourse.mybir as mybir
from concourse.tile import TileContext
from concourse.bass_utils import run_bass_kernel_spmd

dt = mybir.dt
Alu = mybir.AluOpType
Act = mybir.ActivationFunctionType

P = 128
B, N, C = 2, 8000, 512
H, HD = 8, 64
D3 = 20          # full spatial edge (D=H=W)
DR = 10          # reduced spatial edge
NSR = 1000       # DR**3
CT = 4           # C // P
NCHUNK = 2000    # output rows per core
RC = 4           # row chunks per core
RCW = 500        # rows per chunk
MT = 8           # key tiles
MTW = 125        # keys per tile
BW = 512         # PSUM bank width (f32 elems)
SCALE = HD ** -0.5
EPS = 1e-6

_PROGRAM = None
_HOST = None
TRACE = False
LAST_RESULT = None


def _conv_taps(xf, af, rf, w27_sb, negw27_sb, vecs_sb, nc, ct):
    """Depthwise 3x3x3 stride-2 pad-1 conv for one 128-channel tile (fp16).

    The host permutes x's spatial rows into parity-block order: flat index
    (a*4+b*2+c)*1000 + d*100 + h*10 + w  <->  original (2d+a, 2h+b, 2w+c).
    Every tap then reduces to a 2D/3D access pattern. The three taps with
    both dh==-1 and dw==-1 are emitted over the full block with two small
    compensation ops that cancel the row/slice-wrapped reads exactly.

    xf: (p, 8000) fp16 input view; af: (p, 1000) fp16 accumulator;
    rf: (p, 1000) fp16 output (written by the final tap).
    """
    def tap_meta(dd, dh, dw):
        pa, pb, pc = (0 if dd == 0 else 1), (0 if dh == 0 else 1), (0 if dw == 0 else 1)
        Dd, Dh, Dw = (-1 if dd == -1 else 0), (-1 if dh == -1 else 0), (-1 if dw == -1 else 0)
        bb = (pa * 4 + pb * 2 + pc) * 1000
        d0 = 1 if dd == -1 else 0
        t = (dd + 1) * 9 + (dh + 1) * 3 + (dw + 1)
        return bb, Dd, Dh, Dw, d0, t

    def stt(out_ap, in_ap, scal, acc_ap):
        nc.vector.scalar_tensor_tensor(
            out=out_ap, in0=in_ap, scalar=scal, in1=acc_ap,
            op0=Alu.mult, op1=Alu.add)

    def w(t):
        return w27_sb[:, ct, t:t + 1]

    def negw(t):
        return negw27_sb[:, ct, t:t + 1]

    # center tap (0,0,0) + conv bias, full block 0, on the scalar engine
    nc.scalar.activation(af[:, 0:1000], xf[:, 0:1000], Act.Identity,
                         scale=w(13), bias=vecs_sb[:, ct, 1:2])

    taps = [(a, b, c)
            for a in (-1, 0, 1) for b in (-1, 0, 1) for c in (-1, 0, 1)
            if (a, b, c) != (0, 0, 0) and (a, b, c) != (1, 1, 1)]
    for (dd, dh, dw) in taps:
        bb, Dd, Dh, Dw, d0, t = tap_meta(dd, dh, dw)
        dcnt = DR - d0
        i0 = bb + (d0 + Dd) * 100
        ow = af[:, d0 * 100:1000]
        if dh != -1 and dw != -1:
            # case A: contiguous 2D
            i = xf[:, i0:i0 + dcnt * 100]
            stt(ow, i, w(t), ow)
        elif dw == -1 and dh != -1:
            # case B: (d*h merged, w partial) 3D
            o = ow.rearrange("p (x w) -> p x w", w=10)[:, :, 1:10]
            i = xf[:, i0:i0 + dcnt * 100].rearrange("p (x w) -> p x w", w=10)[:, :, 0:9]
            stt(o, i, w(t), o)
        elif dh == -1 and dw != -1:
            # case C: (d, h*w merged) 3D
            o = ow.rearrange("p (d r) -> p d r", r=100)[:, :, 10:100]
            i = xf[:, i0:i0 + dcnt * 100].rearrange("p (d r) -> p d r", r=100)[:, :, 0:90]
            stt(o, i, w(t), o)
        else:
            # case D: dh==-1 and dw==-1 -> extended full-block op + 2 comps
            s = bb + Dd * 100 - 11
            i = xf[:, d0 * 100 + s:1000 + s]
            stt(ow, i, w(t), ow)
            # comp1: out (d, h full, w=0) wrongly read (d, h-1, 9)
            oc1 = ow.rearrange("p (d h w) -> p d h w", h=10, w=10)[:, :, :, 0]
            ic1 = xf[:, d0 * 100 + s:1000 + s].rearrange(
                "p (d h w) -> p d h w", h=10, w=10)[:, :, :, 0]
            stt(oc1, ic1, negw(t), oc1)
            # comp2: out (d, h=0, w 1..9) wrongly read (d-1, 9, w-1)
            oc2 = ow.rearrange("p (d h w) -> p d h w", h=10, w=10)[:, :, 0, 1:10]
            ic2 = xf[:, d0 * 100 + s:1000 + s].rearrange(
                "p (d h w) -> p d h w", h=10, w=10)[:, :, 0, 1:10]
            stt(oc2, ic2, negw(t), oc2)

    # final tap (1,1,1): full block 7, writes the result
    stt(rf[:, 0:1000], xf[:, 7000:8000], w(26), af[:, 0:1000])


def _ln_stats(nc, work, ones1_sb, ones128_sb, x_tiles, sq_tiles, width,
              eps_sb, sx, sxx, muB, rstdB):
    """Cross-partition LayerNorm stats for C=512 split over 4 partition tiles.

    x_tiles/sq_tiles: lists of 4 fp16 APs, each (128, width). sx/sxx are
    (1, width) f32 PSUM APs; muB/rstdB are (128, width) f32 PSUM APs that
    receive the broadcast mean / inverse-std.
    """
    f32, f16 = dt.float32, dt.float16
    n = len(x_tiles)
    for i, xt in enumerate(x_tiles):
        nc.tensor.matmul(sx, ones1_sb[:], xt, start=(i == 0), stop=(i == n - 1))
    for i, st in enumerate(sq_tiles):
        nc.tensor.matmul(sxx, ones1_sb[:], st, start=(i == 0), stop=(i == n - 1))
    mu_r = work.tile([1, width], f16, tag="mu", bufs=2)
    nc.vector.tensor_scalar_mul(out=mu_r[:], in0=sx, scalar1=1.0 / C)
    msq = work.tile([1, width], f32, tag="msq", bufs=2)
    nc.vector.tensor_scalar_mul(out=msq[:], in0=sxx, scalar1=1.0 / C)
    mu2 = work.tile([1, width], f32, tag="mu2", bufs=2)
    nc.vector.tensor_mul(out=mu2[:], in0=mu_r[:], in1=mu_r[:])
    var = work.tile([1, width], f32, tag="var", bufs=2)
    nc.vector.tensor_sub(out=var[:], in0=msq[:], in1=mu2[:])
    std = work.tile([1, width], f32, tag="std", bufs=2)
    nc.scalar.activation(std[:], var[:], Act.Sqrt, bias=eps_sb[0:1, 0:1])
    rstd_r = work.tile([1, width], f16, tag="rstd", bufs=2)
    nc.vector.reciprocal(out=rstd_r[:], in_=std[:])
    nc.tensor.matmul(muB, ones128_sb[:], mu_r[:], start=True, stop=True)
    nc.tensor.matmul(rstdB, ones128_sb[:], rstd_r[:], start=True, stop=True)


def _build_program():
    nc = bacc.Bacc("TRN2", target_bir_lowering=False, debug=False, num_devices=8)
    f32, f16, bf16 = dt.float32, dt.float16, dt.bfloat16

    xqtd = nc.dram_tensor("xqt", [CT, P, NCHUNK], f16, kind="ExternalInput").ap()
    xtd = nc.dram_tensor("xtd", [CT, P, N], bf16, kind="ExternalInput").ap()
    wq = nc.dram_tensor("wq", [C, C], f16, kind="ExternalInput").ap()
    wkv = nc.dram_tensor("wkv", [C, 2 * C], f16, kind="ExternalInput").ap()
    wp = nc.dram_tensor("wp", [C, C], f16, kind="ExternalInput").ap()
    w27d = nc.dram_tensor("w27", [C, 27], f32, kind="ExternalInput").ap()
    vecsd = nc.dram_tensor("vecs", [C, 7], f32, kind="ExternalInput").ap()
    bkvd = nc.dram_tensor("bkv", [2 * C], f32, kind="ExternalInput").ap()
    utd = nc.dram_tensor("ut", [NSR, NCHUNK], f16, kind="ExternalInput").ap()
    eyed = nc.dram_tensor("eye", [P, P], f16, kind="ExternalInput").ap()
    e8d = nc.dram_tensor("e8", [H, C], f16, kind="ExternalInput").ap()
    ones1d = nc.dram_tensor("ones1", [P, 1], f16, kind="ExternalInput").ap()
    ones128d = nc.dram_tensor("ones128", [1, P], f16, kind="ExternalInput").ap()
    epsd = nc.dram_tensor("epsv", [P, 1], f32, kind="ExternalInput").ap()
    yt = nc.dram_tensor("yt", [C, NCHUNK], f16, kind="ExternalOutput").ap()

    with TileContext(nc) as tc, nc.allow_low_precision(
            reason="fp16 data with fp32 PSUM accumulation; tol is 2e-2"):
        with contextlib.ExitStack() as octx:
            consts = octx.enter_context(tc.tile_pool(name="consts", bufs=1))
            keep1 = octx.enter_context(tc.tile_pool(name="keep1", bufs=1))
            work = octx.enter_context(tc.tile_pool(name="work", bufs=2))

            # ---------- constants ----------
            eye_sb = consts.tile([P, P], f16)
            nc.sync.dma_start(out=eye_sb[:], in_=eyed[:])
            e8_sb = consts.tile([H, C], f16)
            nc.sync.dma_start(out=e8_sb[:], in_=e8d[:])
            ones1_sb = consts.tile([P, 1], f16)
            nc.sync.dma_start(out=ones1_sb[:], in_=ones1d[:])
            ones128_sb = consts.tile([1, P], f16)
            nc.sync.dma_start(out=ones128_sb[:], in_=ones128d[:])
            eps_sb = consts.tile([P, 1], f32)
            nc.sync.dma_start(out=eps_sb[:], in_=epsd[:])
            w27_sb = consts.tile([P, CT, 27], f32)
            nc.sync.dma_start(out=w27_sb[:], in_=w27d.rearrange("(o p) t -> p o t", p=P))
            vecs_sb = consts.tile([P, CT, 7], f32)
            nc.sync.dma_start(out=vecs_sb[:], in_=vecsd.rearrange("(o p) t -> p o t", p=P))
            bkv_sb = consts.tile([P, 2 * CT], f32)
            nc.sync.dma_start(out=bkv_sb[:], in_=bkvd.rearrange("(o p) -> p o", p=P))
            negw27_sb = consts.tile([P, CT, 27], f32)
            for ct in range(CT):
                nc.vector.tensor_scalar_mul(out=negw27_sb[:, ct, :],
                                            in0=w27_sb[:, ct, :], scalar1=-1.0)

            qT = keep1.tile([P, CT, NCHUNK], f16)       # 16 KB/part
            kT = keep1.tile([P, CT, NSR], f16)          # 8 KB/part
            lnidT = keep1.tile([P, CT, NCHUNK], f16)    # 16 KB/part
            wp_sb = keep1.tile([P, CT, C], f16)
            v_nat = keep1.tile([P, MT, C], f16)
            v_aug = keep1.tile([P, MT, H, HD + 1], f16)

            with contextlib.ExitStack() as ectx:
                psE = ectx.enter_context(tc.tile_pool(name="psE", bufs=2, space="PSUM"))

                with tc.tile_pool(name="cpool", bufs=1) as cpool, \
                        tc.tile_pool(name="wqp", bufs=1) as wqp:
                    # ---------- conv input + weight/xq loads ----------
                    wq_sb = wqp.tile([P, CT, C], f16)
                    nc.sync.dma_start(out=wq_sb[:],
                                      in_=wq.rearrange("(k p) m -> p k m", p=P))
                    xqT = wqp.tile([P, CT, NCHUNK], f16)
                    for ct in range(CT):
                        nc.sync.dma_start(out=xqT[:, ct, :], in_=xqtd[ct, :, :])
                    wkv_sb = wqp.tile([P, CT, 2 * C], f16)
                    nc.sync.dma_start(out=wkv_sb[:],
                                      in_=wkv.rearrange("(k p) m -> p k m", p=P))
                    nc.sync.dma_start(out=wp_sb[:],
                                      in_=wp.rearrange("(k p) m -> p k m", p=P))

                    # ---------- conv + squares, per channel tile (DVE) ----------
                    xr = cpool.tile([P, CT, NSR], f16)
                    sq = cpool.tile([P, CT, NSR], f16)
                    for ct in range(CT):
                        xct = cpool.tile([P, N], bf16, tag="xct", bufs=2)
                        nc.sync.dma_start(out=xct[:], in_=xtd[ct, :, :])
                        acc_t = cpool.tile([P, NSR], bf16, tag="acc", bufs=2)
                        _conv_taps(xct[:], acc_t[:], xr[:, ct, :], w27_sb,
                                   negw27_sb, vecs_sb, nc, ct)
                        nc.scalar.activation(sq[:, ct, :], xr[:, ct, :], Act.Square)

                    # ---------- q projection (PE, overlaps conv) ----------
                    for ct in range(CT):
                        for rc in range(RC):
                            acc = psE.tile([P, RCW], f32, tag="proj")
                            for kt in range(CT):
                                nc.tensor.matmul(
                                    acc[:], wq_sb[:, kt, ct * P:(ct + 1) * P],
                                    xqT[:, kt, rc * RCW:(rc + 1) * RCW],
                                    start=(kt == 0), stop=(kt == CT - 1))
                            nc.scalar.activation(
                                qT[:, ct, rc * RCW:(rc + 1) * RCW], acc[:],
                                Act.Identity, bias=vecs_sb[:, ct, 0:1])

                    # ---------- LayerNorm over C -> xrn ----------
                    xrn = cpool.tile([P, CT, NSR], f16)
                    for ch in range(2):
                        cs = slice(ch * RCW, (ch + 1) * RCW)
                        sx = psE.tile([1, RCW], f32, tag="stat")
                        sxx = psE.tile([1, RCW], f32, tag="stat")
                        muB = psE.tile([P, RCW], f32, tag="bcast")
                        rstdB = psE.tile([P, RCW], f32, tag="bcast")
                        _ln_stats(nc, work, ones1_sb, ones128_sb,
                                  [xr[:, ct, cs] for ct in range(CT)],
                                  [sq[:, ct, cs] for ct in range(CT)], RCW,
                                  eps_sb, sx[:], sxx[:], muB[:], rstdB[:])
                        for ct in range(CT):
                            t1 = work.tile([P, RCW], f32, tag="lnt")
                            nc.vector.tensor_sub(out=t1[:], in0=xr[:, ct, cs],
                                                 in1=muB[:])
                            t2 = work.tile([P, RCW], f32, tag="lnt2")
                            nc.vector.tensor_mul(out=t2[:], in0=t1[:], in1=rstdB[:])
                            nc.vector.tensor_scalar(
                                out=xrn[:, ct, cs], in0=t2[:],
                                scalar1=vecs_sb[:, ct, 2:3], scalar2=vecs_sb[:, ct, 3:4],
                                op0=Alu.mult, op1=Alu.add)

                    # ---------- kv projection ----------
                    vT = cpool.tile([P, CT, NSR], f16)
                    for mt8 in range(2 * CT):
                        dsts = kT if mt8 < CT else vT
                        di = mt8 if mt8 < CT else mt8 - CT
                        for ch in range(2):
                            cs = slice(ch * RCW, (ch + 1) * RCW)
                            acc = psE.tile([P, RCW], f32, tag="proj")
                            for kt in range(CT):
                                nc.tensor.matmul(
                                    acc[:], wkv_sb[:, kt, mt8 * P:(mt8 + 1) * P],
                                    xrn[:, kt, cs],
                                    start=(kt == 0), stop=(kt == CT - 1))
                            nc.vector.tensor_scalar_add(
                                out=dsts[:, di, cs], in0=acc[:],
                                scalar1=bkv_sb[:, mt8:mt8 + 1])

                    # ---------- v natural + ones column (v_aug) ----------
                    nc.gpsimd.tensor_copy(
                        out=v_aug[:, :, :, HD:HD + 1],
                        in_=ones1_sb[:, 0:1, None, None].to_broadcast([P, MT, H, 1]))
                    for ci in range(CT):
                        for mt in range(MT):
                            tp = psE.tile([P, P], f16, tag="trps")
                            nc.tensor.transpose(
                                tp[:MTW, :], vT[:, ci, mt * MTW:(mt + 1) * MTW],
                                eye_sb[:])
                            nc.vector.tensor_copy(
                                out=v_nat[:MTW, mt, ci * P:(ci + 1) * P],
                                in_=tp[:MTW, :])
                            nc.vector.tensor_copy(out=v_aug[:MTW, mt, 2 * ci, 0:HD],
                                                  in_=tp[:MTW, 0:HD])
                            nc.vector.tensor_copy(out=v_aug[:MTW, mt, 2 * ci + 1, 0:HD],
                                                  in_=tp[:MTW, HD:2 * HD])

            # ---------- per row-chunk: identity (U matmul) + LN + attention ----
            with contextlib.ExitStack() as actx:
                psA = actx.enter_context(tc.tile_pool(name="psA", bufs=1, space="PSUM"))
                ld2 = actx.enter_context(tc.tile_pool(name="ld2", bufs=2))
                ppool = actx.enter_context(tc.tile_pool(name="ppool", bufs=2))
                apool = actx.enter_context(tc.tile_pool(name="apool", bufs=1))

                for rc in range(RC):
                    rs = slice(rc * RCW, (rc + 1) * RCW)

                    # --- identity branch: idT = v_nat^T @ U^T, then LN ---
                    ut_t = []
                    for mt in range(MT):
                        u1 = ld2.tile([P, RCW], f16, tag="uld", bufs=10)
                        nc.sync.dma_start(out=u1[:MTW, :],
                                          in_=utd[mt * MTW:(mt + 1) * MTW, rs])
                        ut_t.append(u1)
                    idr = apool.tile([P, CT, RCW], f16, tag="idr", bufs=2)
                    idsq = apool.tile([P, CT, RCW], f16, tag="idsq", bufs=2)
                    for cp in range(2):          # ct pairs share a 2-bank tile
                        idp = psA.tile([P, 2, BW], f32, tag="sc", bufs=3)
                        for k in range(2):
                            ct = 2 * cp + k
                            for mt in range(MT):
                                nc.tensor.matmul(
                                    idp[:, k, 0:RCW],
                                    v_nat[:MTW, mt, ct * P:(ct + 1) * P],
                                    ut_t[mt][:MTW, :],
                                    start=(mt == 0), stop=(mt == MT - 1))
                        nc.vector.tensor_copy(out=idr[:, 2 * cp:2 * cp + 2, :],
                                              in_=idp[:, :, 0:RCW])
                        nc.scalar.activation(idsq[:, 2 * cp:2 * cp + 2, :],
                                             idp[:, :, 0:RCW], Act.Square)
                    sxt = psA.tile([P, BW], f32, tag="ov", bufs=2)
                    sxxt = psA.tile([P, BW], f32, tag="ov", bufs=2)
                    mb = psA.tile([P, 2, BW], f32, tag="sc", bufs=3)
                    _ln_stats(nc, work, ones1_sb, ones128_sb,
                              [idr[:, ct, :] for ct in range(CT)],
                              [idsq[:, ct, :] for ct in range(CT)], RCW,
                              eps_sb, sxt[0:1, 0:RCW], sxxt[0:1, 0:RCW],
                              mb[:, 0, 0:RCW], mb[:, 1, 0:RCW])
                    for ct in range(CT):
                        t1 = work.tile([P, RCW], f32, tag="lnt")
                        nc.vector.tensor_sub(out=t1[:], in0=idr[:, ct, :],
                                             in1=mb[:, 0, 0:RCW])
                        t2 = work.tile([P, RCW], f32, tag="lnt2")
                        nc.vector.tensor_mul(out=t2[:], in0=t1[:], in1=mb[:, 1, 0:RCW])
                        nc.vector.tensor_scalar(
                            out=lnidT[:, ct, rs], in0=t2[:],
                            scalar1=vecs_sb[:, ct, 4:5], scalar2=vecs_sb[:, ct, 5:6],
                            op0=Alu.mult, op1=Alu.add)

                    # --- attention ---
                    oT65 = apool.tile([P, H, RCW], f16, tag="ot65", bufs=2)
                    for hh in range(H):
                        pb = HD * (hh % 2)
                        ci = hh // 2
                        pT = ppool.tile([P, MT, BW], f16, tag="pt")
                        ov = psA.tile([P, BW], f32, tag="ov", bufs=2)
                        for g in range(4):
                            sc = psA.tile([P, 2, BW], f32, tag="sc", bufs=3)
                            for k in range(2):
                                mt = 2 * g + k
                                nc.tensor.matmul(
                                    sc[:MTW, k, 0:RCW],
                                    kT[pb:pb + HD, ci, mt * MTW:(mt + 1) * MTW],
                                    qT[pb:pb + HD, ci, rs],
                                    start=True, stop=True)
                            nc.scalar.activation(pT[:MTW, 2 * g:2 * g + 2, :],
                                                 sc[:MTW, :, :], Act.Exp,
                                                 scale=SCALE)
                        for mt in range(MT):
                            nc.tensor.matmul(
                                ov[0:HD + 1, 0:RCW], v_aug[:MTW, mt, hh, :],
                                pT[:MTW, mt, 0:RCW],
                                start=(mt == 0), stop=(mt == MT - 1))
                        nc.vector.tensor_copy(out=oT65[0:HD + 1, hh, :],
                                              in_=ov[0:HD + 1, 0:RCW])

                    # --- normalize + add identity + output projection ---
                    den8 = apool.tile([H, RCW], f16, tag="den8", bufs=2)
                    for hh in range(H):
                        nc.sync.dma_start(out=den8[hh:hh + 1, :],
                                          in_=oT65[HD:HD + 1, hh, :])
                    rec8 = apool.tile([H, RCW], f16, tag="rec8", bufs=2)
                    nc.vector.reciprocal(out=rec8[:], in_=den8[:])
                    sum_r = apool.tile([P, CT, RCW], f16, tag="sumr", bufs=2)
                    for ct in range(CT):
                        recB = psA.tile([P, 2, BW], f32, tag="sc", bufs=3)
                        nc.tensor.matmul(recB[:, 0, 0:RCW],
                                         e8_sb[:, ct * P:(ct + 1) * P],
                                         rec8[:], start=True, stop=True)
                        tmp = work.tile([P, RCW], f32, tag="ntmp")
                        nc.vector.tensor_mul(out=tmp[0:HD, :],
                                             in0=oT65[0:HD, 2 * ct, :],
                                             in1=recB[0:HD, 0, 0:RCW])
                        nc.vector.tensor_mul(out=tmp[HD:P, :],
                                             in0=oT65[0:HD, 2 * ct + 1, :],
                                             in1=recB[HD:P, 0, 0:RCW])
                        nc.vector.tensor_add(out=sum_r[:, ct, :], in0=tmp[:],
                                             in1=lnidT[:, ct, rs])
                    for ct2 in range(CT):
                        fin = psA.tile([P, 2, BW], f32, tag="sc", bufs=3)
                        for kt in range(CT):
                            nc.tensor.matmul(
                                fin[:, 0, 0:RCW],
                                wp_sb[:, kt, ct2 * P:(ct2 + 1) * P],
                                sum_r[:, kt, :],
                                start=(kt == 0), stop=(kt == CT - 1))
                        oF = apool.tile([P, RCW], f16, tag="of", bufs=2)
                        nc.vector.tensor_scalar_add(out=oF[:], in0=fin[:, 0, 0:RCW],
                                                    scalar1=vecs_sb[:, ct2, 6:7])
                        nc.sync.dma_start(out=yt[ct2 * P:(ct2 + 1) * P, rs], in_=oF[:])

    nc.finalize()
    return nc


def _parity_perm():
    perm = np.empty(N, np.int64)
    for a in range(2):
        for b in range(2):
            for c in range(2):
                blk = (a * 4 + b * 2 + c) * NSR
                for d in range(DR):
                    for h in range(DR):
                        for w_ in range(DR):
                            perm[blk + d * 100 + h * 10 + w_] = (
                                (2 * d + a) * 400 + (2 * h + b) * 20 + (2 * w_ + c))
    return perm


def _host_consts():
    eye = np.eye(P, dtype=np.float16)
    e8 = np.zeros((H, C), np.float16)
    for p in range(C):
        hh = 2 * (p // P) + (p % P) // HD
        e8[hh, p] = 1.0
    ones1 = np.ones((P, 1), np.float16)
    ones128 = np.ones((1, P), np.float16)
    epsv = np.full((P, 1), EPS, np.float32)
    return eye, e8, ones1, ones128, epsv


def _interp_1d(n_out, n_in, off):
    out = []
    for i in range(n_out):
        src = (off + i + 0.5) / 2.0 - 0.5
        lo = int(np.floor(src))
        f = src - lo
        lo_c = min(max(lo, 0), n_in - 1)
        hi_c = min(max(lo + 1, 0), n_in - 1)
        out.append(((lo_c, 1.0 - f), (hi_c, f)))
    return out


def _build_ut(j):
    """U^T (NSR, NCHUNK): idT[:, n] = sum_m v_nat[m, :] * UT[m, n], quarter j."""
    ut = np.zeros((NSR, NCHUNK), np.float32)
    d_lo = (j * NCHUNK) // (D3 * D3)
    dmap = _interp_1d(5, DR, d_lo)
    hmap = _interp_1d(D3, DR, 0)
    wmap = _interp_1d(D3, DR, 0)
    for dd in range(5):
        for hh2 in range(D3):
            for ww in range(D3):
                nloc = dd * D3 * D3 + hh2 * D3 + ww
                for (di, dwt) in dmap[dd]:
                    for (hi, hwt) in hmap[hh2]:
                        for (wi, wwt) in wmap[ww]:
                            m = di * DR * DR + hi * DR + wi
                            ut[m, nloc] += dwt * hwt * wwt
    return ut.astype(np.float16)


def kernel(**inputs):
    global _PROGRAM, _HOST, LAST_RESULT
    x = np.asarray(inputs["x"], np.float32)
    Wq = np.asarray(inputs["Wq"], np.float32)
    bq = np.asarray(inputs["bq"], np.float32)
    Wkv = np.asarray(inputs["Wkv"], np.float32)
    bkv_ = np.asarray(inputs["bkv"], np.float32)
    sr_w = np.asarray(inputs["sr_w"], np.float32)
    sr_b = np.asarray(inputs["sr_b"], np.float32)
    sr_g = np.asarray(inputs["sr_g"], np.float32)
    sr_beta = np.asarray(inputs["sr_beta"], np.float32)
    up_g = np.asarray(inputs["up_g"], np.float32)
    up_beta = np.asarray(inputs["up_beta"], np.float32)
    Wp = np.asarray(inputs["Wp"], np.float32)
    bp = np.asarray(inputs["bp"], np.float32)

    if _PROGRAM is None:
        _PROGRAM = _build_program()
    nc = _PROGRAM

    if _HOST is None:
        _HOST = (_host_consts(), [_build_ut(j) for j in range(4)], _parity_perm())
    (eye, e8, ones1, ones128, epsv), uts, perm = _HOST

    w27 = np.ascontiguousarray(sr_w.reshape(C, 27))
    vecs = np.ascontiguousarray(
        np.stack([bq, sr_b, sr_g, sr_beta, up_g, up_beta, bp], axis=1))
    wq16 = np.ascontiguousarray(Wq.astype(np.float16))
    wkv16 = np.ascontiguousarray(Wkv.astype(np.float16))
    wp16 = np.ascontiguousarray(Wp.astype(np.float16))

    xtds, xqts = [], []
    for b in range(B):
        xtds.append(np.ascontiguousarray(
            x[b][perm].T.reshape(CT, P, N).astype(ml_dtypes.bfloat16)))
        xqts.append([np.ascontiguousarray(
            x[b, j * NCHUNK:(j + 1) * NCHUNK].T.reshape(CT, P, NCHUNK)
            .astype(np.float16)) for j in range(4)])

    in_maps = []
    for core in range(8):
        b, j = core // 4, core % 4
        in_maps.append({
            "xtd": xtds[b],
            "xqt": xqts[b][j],
            "wq": wq16, "wkv": wkv16, "wp": wp16,
            "w27": w27, "vecs": vecs, "bkv": bkv_,
            "ut": uts[j],
            "eye": eye, "e8": e8, "ones1": ones1, "ones128": ones128,
            "epsv": epsv,
        })

    res = run_bass_kernel_spmd(nc, in_maps, core_ids=list(range(8)), trace=TRACE)
    LAST_RESULT = res
    out = np.empty((B, N, C), np.float32)
    for core in range(8):
        b, j = core // 4, core % 4
        out[b, j * NCHUNK:(j + 1) * NCHUNK, :] = (
            res.results[core]["yt"].astype(np.float32).T)
    return out
